# revision 3
# baseline (speedup 1.0000x reference)
"""Trainium2 Bass kernel v3: 2-layer GraphSAGE + link-prediction MLP.

vs baseline:
  - supergroup-packed edge buckets (per-(bucket,block) max-packing) -> 8% pad
  - idx/seg streams SBUF-resident (one DMA per layer, no per-call loads)
  - supergroup-major processing; quarter-aligned split AllGathers overlap
    the next supergroups' work
  - pairs: a-side via PE one-hot expansion from the local shard (no gather),
    only the b-side is gathered -> pair gather idx halved
  - layer-1 table is host-computed (node_ids == arange) -> no AllGather #1
"""

import os
import sys

import numpy as np

_TRN_REPO = "/opt/trn_rl_repo"
if _TRN_REPO not in sys.path:
    sys.path.insert(0, _TRN_REPO)

R = 8
D = 128
GSUP = 7
NIDX_TILES = 32  # max tiles per dma_gather call

_TRACE = False
_LAST_EXEC_NS = None
_LAST_RESULTS = None


def _cdiv(a, b):
    return -(-a // b)


def _wrap16(idx_stream):
    m8 = len(idx_stream) // 16
    a = idx_stream.reshape(m8, 16).T
    return np.tile(a, (8, 1)).astype(np.int16)


class Cfg:
    def __init__(self, N):
        assert N % R == 0
        self.N = N
        self.NLOC = N // R
        self.NB = _cdiv(self.NLOC, 128)
        self.SHARD = self.NB * 128
        self.NSG = _cdiv(self.NB, GSUP)
        qs = []
        rem = self.NSG
        while rem > 0:
            take = rem
            while take * GSUP * 128 * R > 32600:
                take -= 1
            assert take >= 1
            qs.append(take)
            rem -= take
        self.QSG = qs
        self.NQ = len(qs)
        self.sg_q = []
        for qi, n in enumerate(qs):
            self.sg_q += [qi] * n
        self.qrows = [n * GSUP * 128 for n in qs]
        # clip last quarter to SHARD
        tot = sum(self.qrows)
        if tot > self.SHARD:
            self.qrows[-1] -= tot - self.SHARD
        self.qoff = np.concatenate([[0], np.cumsum(self.qrows)]).astype(np.int64)
        assert self.qoff[-1] == self.SHARD


# ------------------------------------------------------------------ planning
def _pack_buckets(cfg, core, q_of, blk, slot_or_aidx, idx_vals, outer_keys,
                  per_tile_flags):
    """Shared bucket packer for edges (outer_keys=(sg,q)) and pairs ((q,))."""
    raise NotImplementedError


def _plan_edges(cfg, senders, receivers):
    N, NLOC = cfg.N, cfg.NLOC
    s = np.asarray(senders, dtype=np.int64)
    r = np.asarray(receivers, dtype=np.int64)
    core = r // NLOC
    sl = s % NLOC
    q_of_e = np.searchsorted(cfg.qoff, sl, side="right") - 1
    idx_in_chunk = (s // NLOC) * np.asarray(cfg.qrows)[q_of_e] + (
        sl - cfg.qoff[q_of_e]
    )
    rl = r % NLOC
    blk = rl // 128
    slot = rl % 128
    NQ, NB, NSG = cfg.NQ, cfg.NB, cfg.NSG

    key = (core * NQ + q_of_e) * NB + blk
    cnt = np.bincount(key, minlength=R * NQ * NB).reshape(R, NQ, NB)
    mcnt = cnt.max(axis=0)  # [NQ, NB]

    buckets = []
    TT = 0
    TMM = 0
    boff = {}
    for g in range(NSG):
        blks = list(range(g * GSUP, min((g + 1) * GSUP, NB)))
        sg_mms = []
        for q in range(NQ):
            offs = {}
            off = 0
            for b in blks:
                offs[b] = off
                off += int(mcnt[q, b])
            ntiles = _cdiv(off, 128) if off else 0
            mms = []
            for b in blks:
                lo = offs[b]
                hi = lo + int(mcnt[q, b])
                if hi == lo:
                    continue
                t0, t1 = lo // 128, (hi - 1) // 128
                for t in range(t0, t1 + 1):
                    mms.append(
                        dict(tile=t, blk=b, bank=b - g * GSUP,
                             mi=TMM + len(mms), stop=False)
                    )
                boff[(q, b)] = (len(buckets), lo)
            buckets.append(dict(sg=g, q=q, blks=blks, offs=offs, ntiles=ntiles,
                                tile0=TT, mm0=TMM, nmm=len(mms), mms=mms))
            sg_mms.extend(mms)
            TT += ntiles
            TMM += len(mms)
        # global stop flags: last mm per block across the sg's quarters
        last = {}
        for mm in sg_mms:
            last[mm["blk"]] = mm
        for mm in last.values():
            mm["stop"] = True
        for q in range(NQ):
            bk = buckets[-(NQ - q)]
            bk["blocks_with_mms"] = set(last.keys())

    eidx = np.zeros((R, max(TT, 1) * 128), np.int64)
    eseg = np.full((R, max(TMM, 1), 128), -1.0, np.float32)
    order = np.lexsort((slot, blk, q_of_e, core))
    ks_s = key[order]
    uniq, starts = np.unique(ks_s, return_index=True)
    starts = list(starts) + [len(order)]
    for ui, kk in enumerate(uniq):
        k0, k1 = starts[ui], starts[ui + 1]
        ei = order[k0:k1]
        kcore = int(kk) // (NQ * NB)
        rem = int(kk) % (NQ * NB)
        qq, bb = rem // NB, rem % NB
        bi, lo = boff[(qq, bb)]
        bk = buckets[bi]
        base = bk["tile0"] * 128 + lo
        n = k1 - k0
        eidx[kcore, base : base + n] = idx_in_chunk[ei]
        for mm in bk["mms"]:
            if mm["blk"] != bb:
                continue
            t = mm["tile"]
            row_lo, row_hi = t * 128, (t + 1) * 128
            a = max(lo, row_lo)
            bnd = min(lo + n, row_hi)
            if bnd > a:
                rows = np.arange(a, bnd)
                eseg[kcore, mm["mi"], rows - row_lo] = slot[ei[a - lo : bnd - lo]]

    eidx_w = np.zeros((R, 128, max(TT, 1) * 8), np.int16)
    for k in range(R):
        eidx_w[k] = _wrap16(eidx[k].astype(np.int16))
    import ml_dtypes
    eseg_t = np.ascontiguousarray(eseg.transpose(0, 2, 1)).astype(ml_dtypes.bfloat16)

    static = dict(buckets=buckets, TT=TT, TMM=TMM)
    return static, dict(eidx=eidx_w, eseg=eseg_t)


def _plan_pairs(cfg, pairs):
    N, NLOC = cfg.N, cfg.NLOC
    pa = np.asarray(pairs[:, 0], dtype=np.int64)
    pb = np.asarray(pairs[:, 1], dtype=np.int64)
    core = pa // NLOC
    al = pa % NLOC
    ablk = al // 128
    aidx = al % 128
    bl = pb % NLOC
    q_of_p = np.searchsorted(cfg.qoff, bl, side="right") - 1
    bidx = (pb // NLOC) * np.asarray(cfg.qrows)[q_of_p] + (bl - cfg.qoff[q_of_p])
    NQ, NB = cfg.NQ, cfg.NB

    key = (core * NQ + q_of_p) * NB + ablk
    cnt = np.bincount(key, minlength=R * NQ * NB).reshape(R, NQ, NB)
    mcnt = cnt.max(axis=0)

    buckets = []
    PTT = 0
    PMM = 0
    boff = {}
    for q in range(NQ):
        offs = {}
        off = 0
        for b in range(NB):
            offs[b] = off
            off += int(mcnt[q, b])
        ntiles = _cdiv(off, 128) if off else 0
        mms = []
        for b in range(NB):
            lo = offs[b]
            hi = lo + int(mcnt[q, b])
            if hi == lo:
                continue
            t0, t1 = lo // 128, (hi - 1) // 128
            for t in range(t0, t1 + 1):
                mms.append(dict(tile=t, blk=b, mi=PMM + len(mms)))
            boff[(q, b)] = (len(buckets), lo)
        # ensure every tile has at least one mm (dummy with arow=-1)
        covered = {mm["tile"] for mm in mms}
        for t in range(ntiles):
            if t not in covered:
                mms.append(dict(tile=t, blk=0, mi=PMM + len(mms)))
        mms.sort(key=lambda m: (m["tile"], m["mi"]))
        for i, mm in enumerate(mms):
            mm["mi"] = PMM + i
            mm["start"] = i == 0 or mms[i - 1]["tile"] != mm["tile"]
            mm["stop"] = i == len(mms) - 1 or mms[i + 1]["tile"] != mm["tile"]
        buckets.append(dict(q=q, ntiles=ntiles, tile0=PTT, mm0=PMM,
                            nmm=len(mms), mms=mms))
        PTT += ntiles
        PMM += len(mms)

    pidx = np.zeros((R, max(PTT, 1) * 128), np.int64)
    parow = np.full((R, max(PMM, 1), 128), -1.0, np.float32)
    posmap = np.full((R, max(PTT, 1) * 128), -1, np.int64)
    order = np.lexsort((aidx, ablk, q_of_p, core))
    ks_s = key[order]
    uniq, starts = np.unique(ks_s, return_index=True)
    starts = list(starts) + [len(order)]
    for ui, kk in enumerate(uniq):
        k0, k1 = starts[ui], starts[ui + 1]
        pi = order[k0:k1]
        kcore = int(kk) // (NQ * NB)
        rem = int(kk) % (NQ * NB)
        qq, bb = rem // NB, rem % NB
        bi, lo = boff[(qq, bb)]
        bk = buckets[bi]
        base = bk["tile0"] * 128 + lo
        n = k1 - k0
        pidx[kcore, base : base + n] = bidx[pi]
        posmap[kcore, base : base + n] = pi
        for mm in bk["mms"]:
            if mm["blk"] != bb or "start" not in mm:
                continue
            t = mm["tile"]
            row_lo, row_hi = t * 128, (t + 1) * 128
            a = max(lo, row_lo)
            bnd = min(lo + n, row_hi)
            if bnd > a:
                rows = np.arange(a, bnd)
                parow[kcore, mm["mi"], rows - row_lo] = aidx[pi[a - lo : bnd - lo]]

    pidx_w = np.zeros((R, 128, max(PTT, 1) * 8), np.int16)
    for k in range(R):
        pidx_w[k] = _wrap16(pidx[k].astype(np.int16))
    import ml_dtypes
    parow_s = np.ascontiguousarray(parow.reshape(R, 1, -1)).astype(ml_dtypes.bfloat16)

    static = dict(buckets=buckets, PTT=PTT, PMM=PMM)
    return static, dict(pidx=pidx_w, parow=parow_s, posmap=posmap)


def _norms(cfg, senders, receivers):
    N = cfg.N
    s = np.concatenate([senders, np.arange(N, dtype=np.int64)])
    r = np.concatenate([receivers, np.arange(N, dtype=np.int64)])
    deg = np.bincount(s, minlength=N).astype(np.float64)
    cnt = np.bincount(r, minlength=N).astype(np.float64)
    ssend = (1.0 / np.sqrt(np.maximum(deg, 1.0))).astype(np.float32)
    srecv = (np.maximum(cnt, 1.0) ** -1.5).astype(np.float32)
    return ssend, srecv


def _shard_pad(cfg, v):
    out = np.zeros((R, cfg.SHARD) + v.shape[1:], v.dtype)
    for k in range(R):
        out[k, : cfg.NLOC] = v[k * cfg.NLOC : (k + 1) * cfg.NLOC]
    return out


def _chunkify(cfg, tab_sh):
    """tab_sh [R, SHARD, D] -> list of NQ arrays [R*qrows_q, D]."""
    out = []
    for q in range(cfg.NQ):
        rows = cfg.qrows[q]
        arr = np.zeros((R * rows, tab_sh.shape[2]), tab_sh.dtype)
        for k in range(R):
            arr[k * rows : (k + 1) * rows] = tab_sh[
                k, cfg.qoff[q] : cfg.qoff[q] + rows
            ]
        out.append(arr)
    return out


# ------------------------------------------------------------------ numpy sim
def simulate(cfg, est, pst, edata, pdata, inputs):
    """Execute the device schedule in numpy (per core), mirroring _build."""
    import ml_dtypes

    bf = ml_dtypes.bfloat16
    senders = inputs["senders"].astype(np.int64)
    receivers = inputs["receivers"].astype(np.int64)
    ssend, srecv = _norms(cfg, senders, receivers)
    emb = inputs["emb"].astype(np.float32)
    W1, b1 = inputs["W1"].astype(np.float32), inputs["b1"].astype(np.float32)
    W2, b2 = inputs["W2"].astype(np.float32), inputs["b2"].astype(np.float32)
    Wa, ba = inputs["Wa"].astype(np.float32), inputs["ba"].astype(np.float32)
    Wb, bb = inputs["Wb"].astype(np.float32), inputs["bb"].astype(np.float32)

    xn0 = (emb * ssend[:, None]).astype(bf)
    tab0 = _chunkify(cfg, _shard_pad(cfg, xn0))
    x0l = _shard_pad(cfg, emb)
    xn0l = _shard_pad(cfg, xn0)
    ssend_sh = _shard_pad(cfg, ssend)
    srecv_sh = _shard_pad(cfg, srecv)

    def unwrap(widx):  # [128, T*8] wrapped -> stream
        a = widx[:16]
        return np.ascontiguousarray(a.T).reshape(-1)

    def layer(tabs, xn_local, x_local, W, b, relu_out):
        h_all = np.zeros((R, cfg.SHARD, D), np.float32)
        for k in range(R):
            estream = unwrap(edata["eidx"][k])
            seg = edata["eseg"][k].transpose(1, 0)  # [TMM, 128]
            for bki, bk in enumerate(est["buckets"]):
                g = bk["sg"]
                q = bk["q"]
                if bk["ntiles"] == 0:
                    continue
                t0 = bk["tile0"]
                idxs = estream[t0 * 128 : (t0 + bk["ntiles"]) * 128]
                gat = np.asarray(tabs[q])[idxs].astype(bf)  # [nt*128, D]
                if q == 0:
                    bk["_agg"] = {}
                agg = est["buckets"][bki - q]["_agg"] if q else bk["_agg"]
                if q == 0:
                    for j, bb_ in enumerate(bk["blks"]):
                        agg[bb_] = xn_local[k][bb_ * 128 : (bb_ + 1) * 128].astype(
                            bf
                        ).astype(np.float32)
                for mm in bk["mms"]:
                    ind = (
                        seg[mm["mi"]][:, None] == np.arange(128)[None, :]
                    ).astype(np.float32)  # [row, slot]
                    gt = gat[mm["tile"] * 128 : (mm["tile"] + 1) * 128]
                    agg[mm["blk"]] += ind.T @ gt.astype(np.float32)
                if q == cfg.NQ - 1:
                    for j, bb_ in enumerate(bk["blks"]):
                        xupd = agg[bb_] * srecv_sh[k][
                            bb_ * 128 : (bb_ + 1) * 128, None
                        ]
                        xe = x_local[k][bb_ * 128 : (bb_ + 1) * 128]
                        h = (
                            xe.astype(bf).astype(np.float32) @ W[:D]
                            + xupd.astype(bf).astype(np.float32) @ W[D:]
                            + b
                        )
                        h_all[k, bb_ * 128 : (bb_ + 1) * 128] = h
        return h_all

    h1 = layer(tab0, xn0l, x0l, W1, b1, True)
    h1 = np.maximum(h1, 0.0)
    xn1 = (h1 * ssend_sh[:, :, None]).astype(bf)
    tab1 = [None] * cfg.NQ
    for q in range(cfg.NQ):
        rows = cfg.qrows[q]
        arr = np.zeros((R * rows, D), bf)
        for k in range(R):
            arr[k * rows : (k + 1) * rows] = xn1[k, cfg.qoff[q] : cfg.qoff[q] + rows]
        tab1[q] = arr

    h2 = layer(tab1, xn1, h1, W2, b2, False)
    h2b = h2.astype(bf)
    tab2 = [None] * cfg.NQ
    for q in range(cfg.NQ):
        rows = cfg.qrows[q]
        arr = np.zeros((R * rows, D), bf)
        for k in range(R):
            arr[k * rows : (k + 1) * rows] = h2b[k, cfg.qoff[q] : cfg.qoff[q] + rows]
        tab2[q] = arr

    # pairs
    scores = np.zeros(inputs["pairs"].shape[0], np.float32)
    for k in range(R):
        pstream = unwrap(pdata["pidx"][k])
        arow = pdata["parow"][k].reshape(-1, 128).astype(np.float32)
        pm = pdata["posmap"][k]
        for bk in pst["buckets"]:
            q = bk["q"]
            if bk["ntiles"] == 0:
                continue
            t0 = bk["tile0"]
            idxs = pstream[t0 * 128 : (t0 + bk["ntiles"]) * 128]
            gb = np.asarray(tab2[q])[idxs].astype(np.float32)  # [nt*128, D]
            za = np.zeros((bk["ntiles"] * 128, D), np.float32)
            for mm in bk["mms"]:
                ar = arow[mm["mi"]]
                ex = (
                    np.arange(128)[:, None] == ar[None, :]
                ).astype(np.float32)  # [n, slot]
                hj = h2b[k, mm["blk"] * 128 : (mm["blk"] + 1) * 128].astype(
                    np.float32
                )
                za[mm["tile"] * 128 : (mm["tile"] + 1) * 128] += ex.T @ hj
            z = (za * gb).astype(bf).astype(np.float32)
            t1 = np.maximum(z @ Wa + ba, 0.0).astype(bf).astype(np.float32)
            sc = (t1 @ Wb)[:, 0] + bb[0]
            base = t0 * 128
            val = pm[base : base + bk["ntiles"] * 128]
            msk = val >= 0
            scores[val[msk]] = sc[msk]
    return scores


# ------------------------------------------------------------------ bass build
def _build(cfg, est, pst, bb_val):
    from concourse import bass, mybir, bacc
    import concourse.tile as tile
    from concourse.masks import make_identity

    f32 = mybir.dt.float32
    bf16 = mybir.dt.bfloat16
    i16 = mybir.dt.int16

    TT, TMM = max(est["TT"], 1), max(est["TMM"], 1)
    PTT, PMM = max(pst["PTT"], 1), max(pst["PMM"], 1)
    NB, NSG, NQ = cfg.NB, cfg.NSG, cfg.NQ
    SHARD = cfg.SHARD

    nc = bacc.Bacc(
        "TRN2",
        target_bir_lowering=False,
        debug=False,
        num_devices=R,
        num_swdge_queues=4,
    )

    tab0_q = [
        nc.dram_tensor(f"tab0_{q}", [R * cfg.qrows[q], D], bf16,
                       kind="ExternalInput")
        for q in range(NQ)
    ]
    xn0l_t = nc.dram_tensor("xn0l", [SHARD, D], bf16, kind="ExternalInput")
    x0l_t = nc.dram_tensor("x0l", [SHARD, D], bf16, kind="ExternalInput")
    eidx_t = nc.dram_tensor("eidx", [128, TT * 8], i16, kind="ExternalInput")
    eseg_t = nc.dram_tensor("eseg", [128, TMM], bf16, kind="ExternalInput")
    pidx_t = nc.dram_tensor("pidx", [128, PTT * 8], i16, kind="ExternalInput")
    parow_t = nc.dram_tensor("parow", [1, PMM * 128], bf16, kind="ExternalInput")
    ssend_t = nc.dram_tensor("ssend", [SHARD], f32, kind="ExternalInput")
    srecv_t = nc.dram_tensor("srecv", [SHARD], f32, kind="ExternalInput")
    w1t_t = nc.dram_tensor("w1t", [D, D], f32, kind="ExternalInput")
    w1b_t = nc.dram_tensor("w1b", [D, D], f32, kind="ExternalInput")
    w2t_t = nc.dram_tensor("w2t", [D, D], f32, kind="ExternalInput")
    w2b_t = nc.dram_tensor("w2b", [D, D], f32, kind="ExternalInput")
    wa_t = nc.dram_tensor("wa", [D, D], f32, kind="ExternalInput")
    wb_t = nc.dram_tensor("wb", [D, 1], f32, kind="ExternalInput")
    b1_t = nc.dram_tensor("b1", [1, D], f32, kind="ExternalInput")
    b2_t = nc.dram_tensor("b2", [1, D], f32, kind="ExternalInput")
    ba_t = nc.dram_tensor("ba", [D, 1], f32, kind="ExternalInput")
    iota_in = nc.dram_tensor("iota", [128, 128], f32, kind="ExternalInput")
    iotat_in = nc.dram_tensor("iotat", [128, 128], f32, kind="ExternalInput")
    out_t = nc.dram_tensor("scores", [PTT * 128], f32, kind="ExternalOutput")

    rg = [list(range(R))]
    eq = mybir.AluOpType.is_equal
    amax = mybir.AluOpType.max
    amul = mybir.AluOpType.mult
    aadd = mybir.AluOpType.add

    gq = [0]

    def next_queue():
        q = (gq[0] // 2) % 4
        gq[0] += 1
        return q

    with tile.TileContext(nc) as tc:
        with (
            tc.tile_pool(name="const", bufs=1) as cp,
            tc.tile_pool(name="dram", bufs=1, space="DRAM") as dp,
        ):
            def load_bf(src):
                tmp = cp.tile(list(src.shape), f32, name=f"tmp_{src.name}")
                nc.sync.dma_start(tmp[:, :], src[:, :])
                t = cp.tile(list(src.shape), bf16, name=f"bf_{src.name}")
                nc.vector.tensor_copy(t[:, :], tmp[:, :])
                return t

            w1tt, w1bt = load_bf(w1t_t), load_bf(w1b_t)
            w2tt, w2bt = load_bf(w2t_t), load_bf(w2b_t)
            wab, wbb = load_bf(wa_t), load_bf(wb_t)
            b1bt, b2bt = load_bf(b1_t), load_bf(b2_t)
            bat = cp.tile([D, 1], f32)
            nc.sync.dma_start(bat[:, :], ba_t[:, :])

            iota = cp.tile([128, 128], f32)
            nc.sync.dma_start(iota[:, :], iota_in[:, :])
            iotat = cp.tile([128, 128], f32)
            nc.sync.dma_start(iotat[:, :], iotat_in[:, :])
            ones1 = cp.tile([1, 128], bf16)
            nc.vector.memset(ones1[:, :], 1.0)
            ident = cp.tile([128, 128], f32)
            make_identity(nc, ident[:, :])
            identb = cp.tile([128, 128], bf16)
            nc.vector.tensor_copy(identb[:, :], ident[:, :])
            iotab = cp.tile([128, 128], bf16)
            nc.vector.tensor_copy(iotab[:, :], iota[:, :])

            eidx = cp.tile([128, TT * 8], i16, name="eidx")
            nc.sync.dma_start(eidx[:, :], eidx_t[:, :])
            eseg = cp.tile([128, TMM], bf16, name="eseg")
            nc.sync.dma_start(eseg[:, :], eseg_t[:, :])
            pidx = cp.tile([128, PTT * 8], i16, name="pidx")
            nc.sync.dma_start(pidx[:, :], pidx_t[:, :])
            ssend = cp.tile([128, NB], f32, name="ssend")
            nc.sync.dma_start(ssend[:, :], ssend_t[:].rearrange("(b p) -> p b", p=128))
            srecv = cp.tile([128, NB], f32, name="srecv")
            nc.sync.dma_start(srecv[:, :], srecv_t[:].rearrange("(b p) -> p b", p=128))

            agin1 = dp.tile([SHARD, D], bf16)
            h1l = dp.tile([SHARD, D], bf16)
            agin2 = dp.tile([SHARD, D], bf16)
            tab1_q = [
                dp.tile([R * cfg.qrows[q], D], bf16, addr_space="Shared",
                        name=f"tab1_{q}")
                for q in range(NQ)
            ]
            tab2_q = [
                dp.tile([R * cfg.qrows[q], D], bf16, addr_space="Shared",
                        name=f"tab2_{q}")
                for q in range(NQ)
            ]

            def emit_layer(tabs, xn_local, x_local, wtop, wbot, bias, relu,
                           h_out, agin_out, ag_out):
                with (
                    tc.tile_pool(name="gat", bufs=4) as gp,
                    tc.tile_pool(name="ind", bufs=6) as ip,
                    tc.tile_pool(name="epi", bufs=3) as ep,
                    tc.tile_pool(name="agg", bufs=GSUP, space="PSUM") as aggp,
                    tc.tile_pool(name="trh", bufs=1, space="PSUM") as trhp,
                ):
                    pending_ag = []

                    def flush_ag():
                        for qi in pending_ag:
                            nc.gpsimd.collective_compute(
                                "AllGather",
                                mybir.AluOpType.bypass,
                                replica_groups=rg,
                                ins=[
                                    agin_out[
                                        int(cfg.qoff[qi]) : int(cfg.qoff[qi])
                                        + cfg.qrows[qi],
                                        :,
                                    ].opt()
                                ],
                                outs=[ag_out[qi][:, :].opt()],
                            )
                        pending_ag.clear()

                    bi = 0
                    for g in range(NSG):
                        blks = list(range(g * GSUP, min((g + 1) * GSUP, NB)))
                        bwm = est["buckets"][bi].get("blocks_with_mms", set())
                        aggt = [
                            aggp.tile([128, 128], f32, tag="aggt", name=f"agg{j}")
                            for j in range(len(blks))
                        ]
                        for j, b in enumerate(blks):
                            xnb = ep.tile([128, D], bf16, tag="xnb")
                            nc.sync.dma_start(
                                xnb[:, :], xn_local[b * 128 : (b + 1) * 128, :]
                            )
                            nc.tensor.matmul(
                                aggt[j][:, :], lhsT=identb[:, :], rhs=xnb[:, :],
                                start=True, stop=(b not in bwm),
                            )
                        for q in range(NQ):
                            bk = est["buckets"][bi]
                            assert bk["sg"] == g and bk["q"] == q
                            bi += 1
                            nt = bk["ntiles"]
                            if nt == 0:
                                continue
                            t0 = bk["tile0"]
                            gats = []
                            pos = 0
                            while pos < nt:
                                m = min(NIDX_TILES, nt - pos)
                                gat = gp.tile([128, NIDX_TILES * 128], bf16,
                                              tag="gat")
                                nc.gpsimd.dma_gather(
                                    gat[:, : m * 128].rearrange(
                                        "p (t d) -> p t d", d=128
                                    ),
                                    tabs[q][:, :],
                                    eidx[:, (t0 + pos) * 8 : (t0 + pos + m) * 8],
                                    m * 128,
                                    m * 128,
                                    D,
                                    single_packet=False,
                                    queue_num=next_queue(),
                                )
                                gats.append((pos, m, gat))
                                pos += m
                            if q == NQ - 1 and ag_out is not None:
                                flush_ag()
                            for mm in bk["mms"]:
                                t = mm["tile"]
                                gat = None
                                for (p0, m, gg) in gats:
                                    if p0 <= t < p0 + m:
                                        gat = gg[
                                            :, (t - p0) * 128 : (t - p0 + 1) * 128
                                        ]
                                        break
                                ind = ip.tile([128, 128], bf16, tag="ind")
                                nc.vector.tensor_tensor(
                                    out=ind[:, :],
                                    in0=eseg[
                                        :, mm["mi"] : mm["mi"] + 1
                                    ].to_broadcast([128, 128]),
                                    in1=iotab[:, :],
                                    op=eq,
                                )
                                nc.tensor.matmul(
                                    aggt[mm["bank"]][:, :],
                                    lhsT=ind[:, :],
                                    rhs=gat,
                                    start=False,
                                    stop=mm["stop"],
                                )
                        relu_f = mybir.ActivationFunctionType.Relu
                        copy_f = mybir.ActivationFunctionType.Copy
                        for j, b in enumerate(blks):
                            xupd = ep.tile([128, D], bf16, tag="xupd")
                            nc.vector.tensor_scalar_mul(
                                xupd[:, :], aggt[j][:, :], srecv[:, b : b + 1]
                            )
                            ps1 = trhp.tile([128, 128], bf16, tag="trh")
                            nc.tensor.transpose(ps1[:, :], xupd[:, :], identb[:, :])
                            xupdT = ep.tile([128, D], bf16, tag="xupdT")
                            nc.scalar.copy(xupdT[:, :], ps1[:, :])
                            xe = ep.tile([128, D], bf16, tag="xe")
                            nc.sync.dma_start(
                                xe[:, :], x_local[b * 128 : (b + 1) * 128, :]
                            )
                            ps2 = trhp.tile([128, 128], bf16, tag="trh")
                            nc.tensor.transpose(ps2[:, :], xe[:, :], identb[:, :])
                            xT = ep.tile([128, D], bf16, tag="xT")
                            nc.scalar.copy(xT[:, :], ps2[:, :])
                            hps = trhp.tile([128, 128], f32, tag="trh")
                            nc.tensor.matmul(hps[:, :], lhsT=xT[:, :],
                                             rhs=wtop[:, :], start=True, stop=False)
                            nc.tensor.matmul(hps[:, :], lhsT=xupdT[:, :],
                                             rhs=wbot[:, :], start=False, stop=False)
                            nc.tensor.matmul(hps[:, :], lhsT=ones1[:, :],
                                             rhs=bias[:, :], start=False, stop=True)
                            if relu:
                                hx = ep.tile([128, D], bf16, tag="hx")
                                nc.scalar.activation(hx[:, :], hps[:, :], relu_f)
                                nc.sync.dma_start(
                                    h_out[b * 128 : (b + 1) * 128, :], hx[:, :]
                                )
                                xn2 = ep.tile([128, D], bf16, tag="xn2")
                                nc.scalar.activation(
                                    xn2[:, :], hps[:, :], relu_f,
                                    scale=ssend[:, b : b + 1],
                                )
                                nc.sync.dma_start(
                                    agin_out[b * 128 : (b + 1) * 128, :], xn2[:, :]
                                )
                            else:
                                hxb = ep.tile([128, D], bf16, tag="hxb")
                                nc.scalar.copy(hxb[:, :], hps[:, :])
                                nc.sync.dma_start(
                                    agin_out[b * 128 : (b + 1) * 128, :], hxb[:, :]
                                )
                        if ag_out is not None:
                            qi = cfg.sg_q[g]
                            if g == max(
                                g2 for g2 in range(NSG) if cfg.sg_q[g2] == qi
                            ):
                                pending_ag.append(qi)
                                if g == NSG - 1:
                                    flush_ag()

            emit_layer(tab0_q, xn0l_t, x0l_t, w1tt, w1bt, b1bt, True,
                       h1l, agin1, tab1_q)
            emit_layer(tab1_q, agin1, h1l, w2tt, w2bt, b2bt, False,
                       None, agin2, tab2_q)

            # ---------------- pairs
            with (
                tc.tile_pool(name="ph", bufs=1) as php,
                tc.tile_pool(name="pgat", bufs=4) as pgp,
                tc.tile_pool(name="pex", bufs=4) as pxp,
                tc.tile_pool(name="pz", bufs=3) as pzp,
                tc.tile_pool(name="pepi", bufs=3) as pep,
                tc.tile_pool(name="par", bufs=6) as parp,
                tc.tile_pool(name="pza", bufs=3, space="PSUM") as zap,
                tc.tile_pool(name="pzt", bufs=1, space="PSUM") as ztp,
                tc.tile_pool(name="pmm", bufs=1, space="PSUM") as mmp,
                tc.tile_pool(name="psc", bufs=1, space="PSUM") as scp,
                tc.tile_pool(name="parb", bufs=2, space="PSUM") as arbp,
            ):
                h2loc = php.tile([128, NB, D], bf16, name="h2loc")
                nc.sync.dma_start(
                    h2loc[:, :, :],
                    agin2[:, :].rearrange("(b n) f -> n b f", n=128),
                )

                for bk in pst["buckets"]:
                    q = bk["q"]
                    nt = bk["ntiles"]
                    if nt == 0:
                        continue
                    t0 = bk["tile0"]
                    gats = []
                    pos = 0
                    while pos < nt:
                        m = min(NIDX_TILES, nt - pos)
                        gat = pgp.tile([128, NIDX_TILES * 128], bf16, tag="pgat")
                        nc.gpsimd.dma_gather(
                            gat[:, : m * 128].rearrange("p (t d) -> p t d", d=128),
                            tab2_q[q][:, :],
                            pidx[:, (t0 + pos) * 8 : (t0 + pos + m) * 8],
                            m * 128,
                            m * 128,
                            D,
                            single_packet=False,
                            queue_num=next_queue(),
                        )
                        gats.append((pos, m, gat))
                        pos += m
                    ztiles = {}
                    ARB = 16
                    abf = None
                    for li, mm in enumerate(bk["mms"]):
                        t = mm["tile"]
                        if t not in ztiles:
                            ztiles[t] = zap.tile([128, 128], f32, tag="pza", name=f"pza_{t}")
                        if li % ARB == 0:
                            n = min(ARB, bk["nmm"] - li)
                            abf = parp.tile([1, ARB * 128], bf16, tag="par",
                                            name="parbuf")
                            mi0 = bk["mm0"] + li
                            nc.sync.dma_start(
                                abf[:, : n * 128],
                                parow_t[:, mi0 * 128 : (mi0 + n) * 128],
                            )
                        lo = (li % ARB) * 128
                        ar_ps = arbp.tile([128, 128], f32, tag="parp")
                        nc.tensor.matmul(
                            ar_ps[:, :], lhsT=ones1[:, :],
                            rhs=abf[:, lo : lo + 128],
                            start=True, stop=True,
                        )
                        ex = pxp.tile([128, 128], bf16, tag="pex")
                        nc.vector.tensor_tensor(
                            out=ex[:, :],
                            in0=iotat[:, :1].to_broadcast([128, 128]),
                            in1=ar_ps[:, :],
                            op=eq,
                        )
                        nc.tensor.matmul(
                            ztiles[t][:, :],
                            lhsT=ex[:, :],
                            rhs=h2loc[:, mm["blk"], :],
                            start=mm["start"],
                            stop=mm["stop"],
                        )
                    for base in range(0, nt, 4):
                        nb_ = min(4, nt - base)
                        zt_ps = ztp.tile([128, 512], bf16, tag="pzt")
                        zsb = pzp.tile([128, 512], bf16, tag="pz")
                        for i in range(nb_):
                            t = base + i
                            gat = None
                            for (p0, m, gg) in gats:
                                if p0 <= t < p0 + m:
                                    gat = gg[:, (t - p0) * 128 : (t - p0 + 1) * 128]
                                    break
                            nc.vector.tensor_mul(
                                zsb[:, i * 128 : (i + 1) * 128],
                                ztiles[t][:, :],
                                gat,
                            )
                        for i in range(nb_):
                            nc.tensor.matmul(
                                zt_ps[:, i * 128 : (i + 1) * 128],
                                lhsT=zsb[:, i * 128 : (i + 1) * 128],
                                rhs=identb[:, :],
                                is_transpose=True,
                                start=(i == 0),
                                stop=(i == nb_ - 1),
                            )
                        zt = pep.tile([128, 512], bf16, tag="pzt_s")
                        nc.scalar.copy(zt[:, : nb_ * 128], zt_ps[:, : nb_ * 128])
                        za_ps = mmp.tile([128, 512], f32, tag="pmm")
                        for i in range(nb_):
                            nc.tensor.matmul(
                                za_ps[:, i * 128 : (i + 1) * 128],
                                lhsT=wab[:, :],
                                rhs=zt[:, i * 128 : (i + 1) * 128],
                                start=(i == 0),
                                stop=(i == nb_ - 1),
                            )
                        za = pep.tile([128, 512], bf16, tag="pza_s")
                        nc.scalar.activation(
                            za[:, : nb_ * 128], za_ps[:, : nb_ * 128],
                            mybir.ActivationFunctionType.Relu, bias=bat[:, :],
                        )
                        sc_ps = scp.tile([1, 512], f32, tag="psc")
                        for i in range(nb_):
                            nc.tensor.matmul(
                                sc_ps[:, i * 128 : (i + 1) * 128],
                                lhsT=wbb[:, :],
                                rhs=za[:, i * 128 : (i + 1) * 128],
                                start=(i == 0),
                                stop=(i == nb_ - 1),
                            )
                        sc = pep.tile([1, 512], f32, tag="psc_s")
                        nc.scalar.activation(
                            sc[:, : nb_ * 128], sc_ps[:, : nb_ * 128],
                            mybir.ActivationFunctionType.Identity, bias=float(bb_val),
                        )
                        o0 = (t0 + base) * 128
                        nc.sync.dma_start(
                            out_t[o0 : o0 + nb_ * 128].rearrange("(x n) -> x n", x=1),
                            sc[:, : nb_ * 128],
                        )
    nc.compile()
    return nc


# ------------------------------------------------------------------ entry
def kernel(node_ids, senders, receivers, pairs, emb, W1, b1, W2, b2, Wa, ba,
           Wb, bb):
    global _LAST_EXEC_NS, _LAST_RESULTS
    import ml_dtypes
    from concourse import bass_utils

    bf = ml_dtypes.bfloat16
    node_ids = np.asarray(node_ids).astype(np.int64)
    senders = np.asarray(senders).astype(np.int64)
    receivers = np.asarray(receivers).astype(np.int64)
    pairs_np = np.asarray(pairs).astype(np.int64)
    emb = np.asarray(emb, dtype=np.float32)
    W1 = np.asarray(W1, dtype=np.float32)
    b1 = np.asarray(b1, dtype=np.float32)
    W2 = np.asarray(W2, dtype=np.float32)
    b2 = np.asarray(b2, dtype=np.float32)
    Wa = np.asarray(Wa, dtype=np.float32)
    ba = np.asarray(ba, dtype=np.float32)
    Wb = np.asarray(Wb, dtype=np.float32)
    bb = np.asarray(bb, dtype=np.float32)

    N = emb.shape[0]
    cfg = Cfg(N)
    x0 = emb[node_ids]

    est, edata = _plan_edges(cfg, senders, receivers)
    pst, pdata = _plan_pairs(cfg, pairs_np)
    ssend, srecv = _norms(cfg, senders, receivers)

    xn0 = (x0 * ssend[:, None]).astype(bf)
    tab0 = _chunkify(cfg, _shard_pad(cfg, xn0))
    xn0l = _shard_pad(cfg, xn0)
    x0l = _shard_pad(cfg, x0.astype(bf))
    ssend_sh = _shard_pad(cfg, ssend)
    srecv_sh = _shard_pad(cfg, srecv)

    nc = _build(cfg, est, pst, float(bb.reshape(-1)[0]))

    iota = np.tile(np.arange(128, dtype=np.float32), (128, 1))
    iotat = np.ascontiguousarray(iota.T)
    in_maps = []
    for k in range(R):
        in_maps.append(
            dict(
                **{f"tab0_{q}": tab0[q] for q in range(cfg.NQ)},
                xn0l=xn0l[k],
                x0l=x0l[k],
                eidx=edata["eidx"][k],
                eseg=edata["eseg"][k],
                pidx=pdata["pidx"][k],
                parow=pdata["parow"][k],
                ssend=ssend_sh[k],
                srecv=srecv_sh[k],
                w1t=np.ascontiguousarray(W1[:D]),
                w1b=np.ascontiguousarray(W1[D:]),
                w2t=np.ascontiguousarray(W2[:D]),
                w2b=np.ascontiguousarray(W2[D:]),
                wa=Wa,
                wb=Wb,
                b1=b1.reshape(1, D),
                b2=b2.reshape(1, D),
                ba=ba.reshape(D, 1),
                iota=iota,
                iotat=iotat,
            )
        )

    res = bass_utils.run_bass_kernel_spmd(
        nc, in_maps, core_ids=list(range(R)), trace=_TRACE
    )
    _LAST_EXEC_NS = res.exec_time_ns
    _LAST_RESULTS = res

    P = pairs_np.shape[0]
    scores = np.zeros(P, np.float32)
    for k in range(R):
        v = np.asarray(res.results[k]["scores"])
        pm = pdata["posmap"][k]
        m = pm >= 0
        scores[pm[m]] = v[m]
    return scores


# revision 4
# speedup vs baseline: 1.2652x; 1.2652x over previous
"""Trainium2 Bass kernel v3: 2-layer GraphSAGE + link-prediction MLP.

vs baseline:
  - supergroup-packed edge buckets (per-(bucket,block) max-packing) -> 8% pad
  - idx/seg streams SBUF-resident (one DMA per layer, no per-call loads)
  - supergroup-major processing; quarter-aligned split AllGathers overlap
    the next supergroups' work
  - pairs: a-side via PE one-hot expansion from the local shard (no gather),
    only the b-side is gathered -> pair gather idx halved
  - layer-1 table is host-computed (node_ids == arange) -> no AllGather #1
"""

import os
import sys

import numpy as np

_TRN_REPO = "/opt/trn_rl_repo"
if _TRN_REPO not in sys.path:
    sys.path.insert(0, _TRN_REPO)

R = 8
D = 128
GSUP = 7
NIDX_TILES = 32  # max tiles per dma_gather call

_TRACE = False
_LAST_EXEC_NS = None
_LAST_RESULTS = None


def _cdiv(a, b):
    return -(-a // b)


def _wrap16(idx_stream):
    m8 = len(idx_stream) // 16
    a = idx_stream.reshape(m8, 16).T
    return np.tile(a, (8, 1)).astype(np.int16)


class Cfg:
    def __init__(self, N):
        assert N % R == 0
        self.N = N
        self.NLOC = N // R
        self.NB = _cdiv(self.NLOC, 128)
        self.SHARD = self.NB * 128
        self.NSG = _cdiv(self.NB, GSUP)
        qs = []
        rem = self.NSG
        while rem > 0:
            take = rem
            while take * GSUP * 128 * R > 32600:
                take -= 1
            assert take >= 1
            qs.append(take)
            rem -= take
        self.QSG = qs
        self.NQ = len(qs)
        self.sg_q = []
        for qi, n in enumerate(qs):
            self.sg_q += [qi] * n
        self.qrows = [n * GSUP * 128 for n in qs]
        # clip last quarter to SHARD
        tot = sum(self.qrows)
        if tot > self.SHARD:
            self.qrows[-1] -= tot - self.SHARD
        self.qoff = np.concatenate([[0], np.cumsum(self.qrows)]).astype(np.int64)
        assert self.qoff[-1] == self.SHARD


# ------------------------------------------------------------------ planning
def _pack_buckets(cfg, core, q_of, blk, slot_or_aidx, idx_vals, outer_keys,
                  per_tile_flags):
    """Shared bucket packer for edges (outer_keys=(sg,q)) and pairs ((q,))."""
    raise NotImplementedError


def _plan_edges(cfg, senders, receivers):
    N, NLOC = cfg.N, cfg.NLOC
    s = np.asarray(senders, dtype=np.int64)
    r = np.asarray(receivers, dtype=np.int64)
    core = r // NLOC
    sl = s % NLOC
    q_of_e = np.searchsorted(cfg.qoff, sl, side="right") - 1
    idx_in_chunk = (s // NLOC) * np.asarray(cfg.qrows)[q_of_e] + (
        sl - cfg.qoff[q_of_e]
    )
    rl = r % NLOC
    blk = rl // 128
    slot = rl % 128
    NQ, NB, NSG = cfg.NQ, cfg.NB, cfg.NSG

    key = (core * NQ + q_of_e) * NB + blk
    cnt = np.bincount(key, minlength=R * NQ * NB).reshape(R, NQ, NB)
    mcnt = cnt.max(axis=0)  # [NQ, NB]

    buckets = []
    TT = 0
    TMM = 0
    boff = {}
    for g in range(NSG):
        blks = list(range(g * GSUP, min((g + 1) * GSUP, NB)))
        sg_mms = []
        for q in range(NQ):
            offs = {}
            off = 0
            for b in blks:
                offs[b] = off
                off += int(mcnt[q, b])
            ntiles = _cdiv(off, 128) if off else 0
            mms = []
            for b in blks:
                lo = offs[b]
                hi = lo + int(mcnt[q, b])
                if hi == lo:
                    continue
                t0, t1 = lo // 128, (hi - 1) // 128
                for t in range(t0, t1 + 1):
                    mms.append(
                        dict(tile=t, blk=b, bank=b - g * GSUP,
                             mi=TMM + len(mms), stop=False)
                    )
                boff[(q, b)] = (len(buckets), lo)
            buckets.append(dict(sg=g, q=q, blks=blks, offs=offs, ntiles=ntiles,
                                tile0=TT, mm0=TMM, nmm=len(mms), mms=mms))
            sg_mms.extend(mms)
            TT += ntiles
            TMM += len(mms)
        # global stop flags: last mm per block across the sg's quarters
        last = {}
        for mm in sg_mms:
            last[mm["blk"]] = mm
        for mm in last.values():
            mm["stop"] = True
        for q in range(NQ):
            bk = buckets[-(NQ - q)]
            bk["blocks_with_mms"] = set(last.keys())

    eidx = np.zeros((R, max(TT, 1) * 128), np.int64)
    eseg = np.full((R, max(TMM, 1), 128), -1.0, np.float32)
    order = np.lexsort((slot, blk, q_of_e, core))
    ks_s = key[order]
    uniq, starts = np.unique(ks_s, return_index=True)
    starts = list(starts) + [len(order)]
    for ui, kk in enumerate(uniq):
        k0, k1 = starts[ui], starts[ui + 1]
        ei = order[k0:k1]
        kcore = int(kk) // (NQ * NB)
        rem = int(kk) % (NQ * NB)
        qq, bb = rem // NB, rem % NB
        bi, lo = boff[(qq, bb)]
        bk = buckets[bi]
        base = bk["tile0"] * 128 + lo
        n = k1 - k0
        eidx[kcore, base : base + n] = idx_in_chunk[ei]
        for mm in bk["mms"]:
            if mm["blk"] != bb:
                continue
            t = mm["tile"]
            row_lo, row_hi = t * 128, (t + 1) * 128
            a = max(lo, row_lo)
            bnd = min(lo + n, row_hi)
            if bnd > a:
                rows = np.arange(a, bnd)
                eseg[kcore, mm["mi"], rows - row_lo] = slot[ei[a - lo : bnd - lo]]

    eidx_w = np.zeros((R, 128, max(TT, 1) * 8), np.int16)
    for k in range(R):
        eidx_w[k] = _wrap16(eidx[k].astype(np.int16))
    import ml_dtypes
    eseg_t = np.ascontiguousarray(eseg.transpose(0, 2, 1)).astype(ml_dtypes.bfloat16)

    static = dict(buckets=buckets, TT=TT, TMM=TMM)
    return static, dict(eidx=eidx_w, eseg=eseg_t)


def _plan_pairs(cfg, pairs):
    N, NLOC = cfg.N, cfg.NLOC
    pa = np.asarray(pairs[:, 0], dtype=np.int64)
    pb = np.asarray(pairs[:, 1], dtype=np.int64)
    core = pa // NLOC
    al = pa % NLOC
    ablk = al // 128
    aidx = al % 128
    bl = pb % NLOC
    q_of_p = np.searchsorted(cfg.qoff, bl, side="right") - 1
    bidx = (pb // NLOC) * np.asarray(cfg.qrows)[q_of_p] + (bl - cfg.qoff[q_of_p])
    NQ, NB = cfg.NQ, cfg.NB

    key = (core * NQ + q_of_p) * NB + ablk
    cnt = np.bincount(key, minlength=R * NQ * NB).reshape(R, NQ, NB)
    mcnt = cnt.max(axis=0)

    buckets = []
    PTT = 0
    PMM = 0
    boff = {}
    for q in range(NQ):
        offs = {}
        off = 0
        for b in range(NB):
            offs[b] = off
            off += int(mcnt[q, b])
        ntiles = _cdiv(off, 128) if off else 0
        mms = []
        for b in range(NB):
            lo = offs[b]
            hi = lo + int(mcnt[q, b])
            if hi == lo:
                continue
            t0, t1 = lo // 128, (hi - 1) // 128
            for t in range(t0, t1 + 1):
                mms.append(dict(tile=t, blk=b, mi=PMM + len(mms)))
            boff[(q, b)] = (len(buckets), lo)
        # ensure every tile has at least one mm (dummy with arow=-1)
        covered = {mm["tile"] for mm in mms}
        for t in range(ntiles):
            if t not in covered:
                mms.append(dict(tile=t, blk=0, mi=PMM + len(mms)))
        mms.sort(key=lambda m: (m["tile"], m["mi"]))
        for i, mm in enumerate(mms):
            mm["mi"] = PMM + i
            mm["start"] = i == 0 or mms[i - 1]["tile"] != mm["tile"]
            mm["stop"] = i == len(mms) - 1 or mms[i + 1]["tile"] != mm["tile"]
        buckets.append(dict(q=q, ntiles=ntiles, tile0=PTT, mm0=PMM,
                            nmm=len(mms), mms=mms))
        PTT += ntiles
        PMM += len(mms)

    pidx = np.zeros((R, max(PTT, 1) * 128), np.int64)
    parow = np.full((R, max(PMM, 1), 128), -1.0, np.float32)
    posmap = np.full((R, max(PTT, 1) * 128), -1, np.int64)
    order = np.lexsort((aidx, ablk, q_of_p, core))
    ks_s = key[order]
    uniq, starts = np.unique(ks_s, return_index=True)
    starts = list(starts) + [len(order)]
    for ui, kk in enumerate(uniq):
        k0, k1 = starts[ui], starts[ui + 1]
        pi = order[k0:k1]
        kcore = int(kk) // (NQ * NB)
        rem = int(kk) % (NQ * NB)
        qq, bb = rem // NB, rem % NB
        bi, lo = boff[(qq, bb)]
        bk = buckets[bi]
        base = bk["tile0"] * 128 + lo
        n = k1 - k0
        pidx[kcore, base : base + n] = bidx[pi]
        posmap[kcore, base : base + n] = pi
        for mm in bk["mms"]:
            if mm["blk"] != bb or "start" not in mm:
                continue
            t = mm["tile"]
            row_lo, row_hi = t * 128, (t + 1) * 128
            a = max(lo, row_lo)
            bnd = min(lo + n, row_hi)
            if bnd > a:
                rows = np.arange(a, bnd)
                parow[kcore, mm["mi"], rows - row_lo] = aidx[pi[a - lo : bnd - lo]]

    pidx_w = np.zeros((R, 128, max(PTT, 1) * 8), np.int16)
    for k in range(R):
        pidx_w[k] = _wrap16(pidx[k].astype(np.int16))
    import ml_dtypes
    parow_s = np.ascontiguousarray(parow.reshape(R, 1, -1)).astype(ml_dtypes.bfloat16)

    static = dict(buckets=buckets, PTT=PTT, PMM=PMM)
    return static, dict(pidx=pidx_w, parow=parow_s, posmap=posmap)


def _norms(cfg, senders, receivers):
    N = cfg.N
    s = np.concatenate([senders, np.arange(N, dtype=np.int64)])
    r = np.concatenate([receivers, np.arange(N, dtype=np.int64)])
    deg = np.bincount(s, minlength=N).astype(np.float64)
    cnt = np.bincount(r, minlength=N).astype(np.float64)
    ssend = (1.0 / np.sqrt(np.maximum(deg, 1.0))).astype(np.float32)
    srecv = (np.maximum(cnt, 1.0) ** -1.5).astype(np.float32)
    return ssend, srecv


def _shard_pad(cfg, v):
    out = np.zeros((R, cfg.SHARD) + v.shape[1:], v.dtype)
    for k in range(R):
        out[k, : cfg.NLOC] = v[k * cfg.NLOC : (k + 1) * cfg.NLOC]
    return out


def _chunkify(cfg, tab_sh):
    """tab_sh [R, SHARD, D] -> list of NQ arrays [R*qrows_q, D]."""
    out = []
    for q in range(cfg.NQ):
        rows = cfg.qrows[q]
        arr = np.zeros((R * rows, tab_sh.shape[2]), tab_sh.dtype)
        for k in range(R):
            arr[k * rows : (k + 1) * rows] = tab_sh[
                k, cfg.qoff[q] : cfg.qoff[q] + rows
            ]
        out.append(arr)
    return out


# ------------------------------------------------------------------ numpy sim
def simulate(cfg, est, pst, edata, pdata, inputs):
    """Execute the device schedule in numpy (per core), mirroring _build."""
    import ml_dtypes

    bf = ml_dtypes.bfloat16
    senders = inputs["senders"].astype(np.int64)
    receivers = inputs["receivers"].astype(np.int64)
    ssend, srecv = _norms(cfg, senders, receivers)
    emb = inputs["emb"].astype(np.float32)
    W1, b1 = inputs["W1"].astype(np.float32), inputs["b1"].astype(np.float32)
    W2, b2 = inputs["W2"].astype(np.float32), inputs["b2"].astype(np.float32)
    Wa, ba = inputs["Wa"].astype(np.float32), inputs["ba"].astype(np.float32)
    Wb, bb = inputs["Wb"].astype(np.float32), inputs["bb"].astype(np.float32)

    xn0 = (emb * ssend[:, None]).astype(bf)
    tab0 = _chunkify(cfg, _shard_pad(cfg, xn0))
    x0l = _shard_pad(cfg, emb)
    xn0l = _shard_pad(cfg, xn0)
    ssend_sh = _shard_pad(cfg, ssend)
    srecv_sh = _shard_pad(cfg, srecv)

    def unwrap(widx):  # [128, T*8] wrapped -> stream
        a = widx[:16]
        return np.ascontiguousarray(a.T).reshape(-1)

    def layer(tabs, xn_local, x_local, W, b, relu_out):
        h_all = np.zeros((R, cfg.SHARD, D), np.float32)
        for k in range(R):
            estream = unwrap(edata["eidx"][k])
            seg = edata["eseg"][k].transpose(1, 0)  # [TMM, 128]
            for bki, bk in enumerate(est["buckets"]):
                g = bk["sg"]
                q = bk["q"]
                if bk["ntiles"] == 0:
                    continue
                t0 = bk["tile0"]
                idxs = estream[t0 * 128 : (t0 + bk["ntiles"]) * 128]
                gat = np.asarray(tabs[q])[idxs].astype(bf)  # [nt*128, D]
                if q == 0:
                    bk["_agg"] = {}
                agg = est["buckets"][bki - q]["_agg"] if q else bk["_agg"]
                if q == 0:
                    for j, bb_ in enumerate(bk["blks"]):
                        agg[bb_] = xn_local[k][bb_ * 128 : (bb_ + 1) * 128].astype(
                            bf
                        ).astype(np.float32)
                for mm in bk["mms"]:
                    ind = (
                        seg[mm["mi"]][:, None] == np.arange(128)[None, :]
                    ).astype(np.float32)  # [row, slot]
                    gt = gat[mm["tile"] * 128 : (mm["tile"] + 1) * 128]
                    agg[mm["blk"]] += ind.T @ gt.astype(np.float32)
                if q == cfg.NQ - 1:
                    for j, bb_ in enumerate(bk["blks"]):
                        xupd = agg[bb_] * srecv_sh[k][
                            bb_ * 128 : (bb_ + 1) * 128, None
                        ]
                        xe = x_local[k][bb_ * 128 : (bb_ + 1) * 128]
                        h = (
                            xe.astype(bf).astype(np.float32) @ W[:D]
                            + xupd.astype(bf).astype(np.float32) @ W[D:]
                            + b
                        )
                        h_all[k, bb_ * 128 : (bb_ + 1) * 128] = h
        return h_all

    h1 = layer(tab0, xn0l, x0l, W1, b1, True)
    h1 = np.maximum(h1, 0.0)
    xn1 = (h1 * ssend_sh[:, :, None]).astype(bf)
    tab1 = [None] * cfg.NQ
    for q in range(cfg.NQ):
        rows = cfg.qrows[q]
        arr = np.zeros((R * rows, D), bf)
        for k in range(R):
            arr[k * rows : (k + 1) * rows] = xn1[k, cfg.qoff[q] : cfg.qoff[q] + rows]
        tab1[q] = arr

    h2 = layer(tab1, xn1, h1, W2, b2, False)
    h2b = h2.astype(bf)
    tab2 = [None] * cfg.NQ
    for q in range(cfg.NQ):
        rows = cfg.qrows[q]
        arr = np.zeros((R * rows, D), bf)
        for k in range(R):
            arr[k * rows : (k + 1) * rows] = h2b[k, cfg.qoff[q] : cfg.qoff[q] + rows]
        tab2[q] = arr

    # pairs
    scores = np.zeros(inputs["pairs"].shape[0], np.float32)
    for k in range(R):
        pstream = unwrap(pdata["pidx"][k])
        arow = pdata["parow"][k].reshape(-1, 128).astype(np.float32)
        pm = pdata["posmap"][k]
        for bk in pst["buckets"]:
            q = bk["q"]
            if bk["ntiles"] == 0:
                continue
            t0 = bk["tile0"]
            idxs = pstream[t0 * 128 : (t0 + bk["ntiles"]) * 128]
            gb = np.asarray(tab2[q])[idxs].astype(np.float32)  # [nt*128, D]
            za = np.zeros((bk["ntiles"] * 128, D), np.float32)
            for mm in bk["mms"]:
                ar = arow[mm["mi"]]
                ex = (
                    np.arange(128)[:, None] == ar[None, :]
                ).astype(np.float32)  # [n, slot]
                hj = h2b[k, mm["blk"] * 128 : (mm["blk"] + 1) * 128].astype(
                    np.float32
                )
                za[mm["tile"] * 128 : (mm["tile"] + 1) * 128] += ex.T @ hj
            z = (za * gb).astype(bf).astype(np.float32)
            t1 = np.maximum(z @ Wa + ba, 0.0).astype(bf).astype(np.float32)
            sc = (t1 @ Wb)[:, 0] + bb[0]
            base = t0 * 128
            val = pm[base : base + bk["ntiles"] * 128]
            msk = val >= 0
            scores[val[msk]] = sc[msk]
    return scores


# ------------------------------------------------------------------ bass build
def _build(cfg, est, pst, bb_val):
    from concourse import bass, mybir, bacc
    import concourse.tile as tile
    from concourse.masks import make_identity

    f32 = mybir.dt.float32
    bf16 = mybir.dt.bfloat16
    i16 = mybir.dt.int16

    TT, TMM = max(est["TT"], 1), max(est["TMM"], 1)
    PTT, PMM = max(pst["PTT"], 1), max(pst["PMM"], 1)
    NB, NSG, NQ = cfg.NB, cfg.NSG, cfg.NQ
    SHARD = cfg.SHARD

    nc = bacc.Bacc(
        "TRN2",
        target_bir_lowering=False,
        debug=False,
        num_devices=R,
        num_swdge_queues=4,
    )

    tab0_q = [
        nc.dram_tensor(f"tab0_{q}", [R * cfg.qrows[q], D], bf16,
                       kind="ExternalInput")
        for q in range(NQ)
    ]
    xn0l_t = nc.dram_tensor("xn0l", [SHARD, D], bf16, kind="ExternalInput")
    x0l_t = nc.dram_tensor("x0l", [SHARD, D], bf16, kind="ExternalInput")
    eidx_t = nc.dram_tensor("eidx", [128, TT * 8], i16, kind="ExternalInput")
    eseg_t = nc.dram_tensor("eseg", [128, TMM], bf16, kind="ExternalInput")
    pidx_t = nc.dram_tensor("pidx", [128, PTT * 8], i16, kind="ExternalInput")
    parow_t = nc.dram_tensor("parow", [1, PMM * 128], bf16, kind="ExternalInput")
    ssend_t = nc.dram_tensor("ssend", [SHARD], f32, kind="ExternalInput")
    srecv_t = nc.dram_tensor("srecv", [SHARD], f32, kind="ExternalInput")
    w1t_t = nc.dram_tensor("w1t", [D, D], f32, kind="ExternalInput")
    w1b_t = nc.dram_tensor("w1b", [D, D], f32, kind="ExternalInput")
    w2t_t = nc.dram_tensor("w2t", [D, D], f32, kind="ExternalInput")
    w2b_t = nc.dram_tensor("w2b", [D, D], f32, kind="ExternalInput")
    wa_t = nc.dram_tensor("wa", [D, D], f32, kind="ExternalInput")
    wb_t = nc.dram_tensor("wb", [D, 1], f32, kind="ExternalInput")
    b1_t = nc.dram_tensor("b1", [1, D], f32, kind="ExternalInput")
    b2_t = nc.dram_tensor("b2", [1, D], f32, kind="ExternalInput")
    ba_t = nc.dram_tensor("ba", [D, 1], f32, kind="ExternalInput")
    iota_in = nc.dram_tensor("iota", [128, 128], f32, kind="ExternalInput")
    iotat_in = nc.dram_tensor("iotat", [128, 128], f32, kind="ExternalInput")
    out_t = nc.dram_tensor("scores", [PTT * 128], f32, kind="ExternalOutput")

    rg = [list(range(R))]
    eq = mybir.AluOpType.is_equal
    amax = mybir.AluOpType.max
    amul = mybir.AluOpType.mult
    aadd = mybir.AluOpType.add

    gq = [0]

    def next_queue():
        q = (gq[0] // 2) % 4
        gq[0] += 1
        return q

    with tile.TileContext(nc) as tc:
        with (
            tc.tile_pool(name="const", bufs=1) as cp,
            tc.tile_pool(name="dram", bufs=1, space="DRAM") as dp,
        ):
            def load_bf(src):
                tmp = cp.tile(list(src.shape), f32, name=f"tmp_{src.name}")
                nc.sync.dma_start(tmp[:, :], src[:, :])
                t = cp.tile(list(src.shape), bf16, name=f"bf_{src.name}")
                nc.vector.tensor_copy(t[:, :], tmp[:, :])
                return t

            w1tt, w1bt = load_bf(w1t_t), load_bf(w1b_t)
            w2tt, w2bt = load_bf(w2t_t), load_bf(w2b_t)
            wab, wbb = load_bf(wa_t), load_bf(wb_t)
            b1bt, b2bt = load_bf(b1_t), load_bf(b2_t)
            bat = cp.tile([D, 1], f32)
            nc.sync.dma_start(bat[:, :], ba_t[:, :])

            iota = cp.tile([128, 128], f32)
            nc.sync.dma_start(iota[:, :], iota_in[:, :])
            iotat = cp.tile([128, 128], f32)
            nc.sync.dma_start(iotat[:, :], iotat_in[:, :])
            ones1 = cp.tile([1, 128], bf16)
            nc.vector.memset(ones1[:, :], 1.0)
            ident = cp.tile([128, 128], f32)
            make_identity(nc, ident[:, :])
            identb = cp.tile([128, 128], bf16)
            nc.vector.tensor_copy(identb[:, :], ident[:, :])
            iotab = cp.tile([128, 128], bf16)
            nc.vector.tensor_copy(iotab[:, :], iota[:, :])

            eidx = cp.tile([128, TT * 8], i16, name="eidx")
            nc.sync.dma_start(eidx[:, :], eidx_t[:, :])
            eseg = cp.tile([128, TMM], bf16, name="eseg")
            nc.sync.dma_start(eseg[:, :], eseg_t[:, :])
            pidx = cp.tile([128, PTT * 8], i16, name="pidx")
            nc.sync.dma_start(pidx[:, :], pidx_t[:, :])
            ssend = cp.tile([128, NB], f32, name="ssend")
            nc.sync.dma_start(ssend[:, :], ssend_t[:].rearrange("(b p) -> p b", p=128))
            srecv = cp.tile([128, NB], f32, name="srecv")
            nc.sync.dma_start(srecv[:, :], srecv_t[:].rearrange("(b p) -> p b", p=128))

            agin1 = dp.tile([SHARD, D], bf16)
            h1l = dp.tile([SHARD, D], bf16)
            agin2 = dp.tile([SHARD, D], bf16)
            tab1_q = [
                dp.tile([R * cfg.qrows[q], D], bf16, addr_space="Shared",
                        name=f"tab1_{q}")
                for q in range(NQ)
            ]
            tab2_q = [
                dp.tile([R * cfg.qrows[q], D], bf16, addr_space="Shared",
                        name=f"tab2_{q}")
                for q in range(NQ)
            ]

            def emit_layer(tabs, xn_local, x_local, wtop, wbot, bias, relu,
                           h_out, agin_out, ag_out):
                with (
                    tc.tile_pool(name="gat", bufs=4) as gp,
                    tc.tile_pool(name="ind", bufs=4) as ip,
                    tc.tile_pool(name="epi", bufs=3) as ep,
                    tc.tile_pool(name="agg", bufs=GSUP, space="PSUM") as aggp,
                    tc.tile_pool(name="trh", bufs=1, space="PSUM") as trhp,
                ):
                    pending_ag = []

                    def flush_ag():
                        for qi in pending_ag:
                            nc.gpsimd.collective_compute(
                                "AllGather",
                                mybir.AluOpType.bypass,
                                replica_groups=rg,
                                ins=[
                                    agin_out[
                                        int(cfg.qoff[qi]) : int(cfg.qoff[qi])
                                        + cfg.qrows[qi],
                                        :,
                                    ].opt()
                                ],
                                outs=[ag_out[qi][:, :].opt()],
                            )
                        pending_ag.clear()

                    bi = 0
                    for g in range(NSG):
                        blks = list(range(g * GSUP, min((g + 1) * GSUP, NB)))
                        bwm = est["buckets"][bi].get("blocks_with_mms", set())
                        aggt = [
                            aggp.tile([128, 128], f32, tag="aggt", name=f"agg{j}")
                            for j in range(len(blks))
                        ]
                        for j, b in enumerate(blks):
                            xnb = ep.tile([128, D], bf16, tag="xnb")
                            nc.sync.dma_start(
                                xnb[:, :], xn_local[b * 128 : (b + 1) * 128, :]
                            )
                            nc.tensor.matmul(
                                aggt[j][:, :], lhsT=identb[:, :], rhs=xnb[:, :],
                                start=True, stop=(b not in bwm),
                            )
                        for q in range(NQ):
                            bk = est["buckets"][bi]
                            assert bk["sg"] == g and bk["q"] == q
                            bi += 1
                            nt = bk["ntiles"]
                            if nt == 0:
                                continue
                            t0 = bk["tile0"]
                            gats = []
                            pos = 0
                            while pos < nt:
                                m = min(NIDX_TILES, nt - pos)
                                gat = gp.tile([128, NIDX_TILES * 128], bf16,
                                              tag="gat")
                                nc.gpsimd.dma_gather(
                                    gat[:, : m * 128].rearrange(
                                        "p (t d) -> p t d", d=128
                                    ),
                                    tabs[q][:, :],
                                    eidx[:, (t0 + pos) * 8 : (t0 + pos + m) * 8],
                                    m * 128,
                                    m * 128,
                                    D,
                                    single_packet=False,
                                    queue_num=next_queue(),
                                )
                                gats.append((pos, m, gat))
                                pos += m

                            for mm in bk["mms"]:
                                t = mm["tile"]
                                gat = None
                                for (p0, m, gg) in gats:
                                    if p0 <= t < p0 + m:
                                        gat = gg[
                                            :, (t - p0) * 128 : (t - p0 + 1) * 128
                                        ]
                                        break
                                ind = ip.tile([128, 128], bf16, tag="ind")
                                nc.vector.tensor_tensor(
                                    out=ind[:, :],
                                    in0=eseg[
                                        :, mm["mi"] : mm["mi"] + 1
                                    ].to_broadcast([128, 128]),
                                    in1=iotab[:, :],
                                    op=eq,
                                )
                                nc.tensor.matmul(
                                    aggt[mm["bank"]][:, :],
                                    lhsT=ind[:, :],
                                    rhs=gat,
                                    start=False,
                                    stop=mm["stop"],
                                )
                        relu_f = mybir.ActivationFunctionType.Relu
                        copy_f = mybir.ActivationFunctionType.Copy
                        for j, b in enumerate(blks):
                            xupd = ep.tile([128, D], bf16, tag="xupd")
                            nc.vector.tensor_scalar_mul(
                                xupd[:, :], aggt[j][:, :], srecv[:, b : b + 1]
                            )
                            ps1 = trhp.tile([128, 128], bf16, tag="trh")
                            nc.tensor.transpose(ps1[:, :], xupd[:, :], identb[:, :])
                            xupdT = ep.tile([128, D], bf16, tag="xupdT")
                            nc.scalar.copy(xupdT[:, :], ps1[:, :])
                            xe = ep.tile([128, D], bf16, tag="xe")
                            nc.sync.dma_start(
                                xe[:, :], x_local[b * 128 : (b + 1) * 128, :]
                            )
                            ps2 = trhp.tile([128, 128], bf16, tag="trh")
                            nc.tensor.transpose(ps2[:, :], xe[:, :], identb[:, :])
                            xT = ep.tile([128, D], bf16, tag="xT")
                            nc.scalar.copy(xT[:, :], ps2[:, :])
                            hps = trhp.tile([128, 128], f32, tag="trh")
                            nc.tensor.matmul(hps[:, :], lhsT=xT[:, :],
                                             rhs=wtop[:, :], start=True, stop=False)
                            nc.tensor.matmul(hps[:, :], lhsT=xupdT[:, :],
                                             rhs=wbot[:, :], start=False, stop=False)
                            nc.tensor.matmul(hps[:, :], lhsT=ones1[:, :],
                                             rhs=bias[:, :], start=False, stop=True)
                            if relu:
                                hx = ep.tile([128, D], bf16, tag="hx")
                                nc.scalar.activation(hx[:, :], hps[:, :], relu_f)
                                nc.sync.dma_start(
                                    h_out[b * 128 : (b + 1) * 128, :], hx[:, :]
                                )
                                xn2 = ep.tile([128, D], bf16, tag="xn2")
                                nc.scalar.activation(
                                    xn2[:, :], hps[:, :], relu_f,
                                    scale=ssend[:, b : b + 1],
                                )
                                nc.sync.dma_start(
                                    agin_out[b * 128 : (b + 1) * 128, :], xn2[:, :]
                                )
                            else:
                                hxb = ep.tile([128, D], bf16, tag="hxb")
                                nc.scalar.copy(hxb[:, :], hps[:, :])
                                nc.sync.dma_start(
                                    agin_out[b * 128 : (b + 1) * 128, :], hxb[:, :]
                                )
                        if ag_out is not None:
                            qi = cfg.sg_q[g]
                            if g == max(
                                g2 for g2 in range(NSG) if cfg.sg_q[g2] == qi
                            ):
                                pending_ag.append(qi)
                                flush_ag()

            emit_layer(tab0_q, xn0l_t, x0l_t, w1tt, w1bt, b1bt, True,
                       h1l, agin1, tab1_q)
            emit_layer(tab1_q, agin1, h1l, w2tt, w2bt, b2bt, False,
                       None, agin2, tab2_q)

            # ---------------- pairs
            with (
                tc.tile_pool(name="ph", bufs=1) as php,
                tc.tile_pool(name="pgat", bufs=3) as pgp,
                tc.tile_pool(name="pex", bufs=4) as pxp,
                tc.tile_pool(name="pz", bufs=3) as pzp,
                tc.tile_pool(name="pepi", bufs=3) as pep,
                tc.tile_pool(name="par", bufs=6) as parp,
                tc.tile_pool(name="pza", bufs=3, space="PSUM") as zap,
                tc.tile_pool(name="pzt", bufs=1, space="PSUM") as ztp,
                tc.tile_pool(name="pmm", bufs=1, space="PSUM") as mmp,
                tc.tile_pool(name="psc", bufs=1, space="PSUM") as scp,
                tc.tile_pool(name="parb", bufs=2, space="PSUM") as arbp,
            ):
                h2loc = php.tile([128, NB, D], bf16, name="h2loc")
                nc.sync.dma_start(
                    h2loc[:, :, :],
                    agin2[:, :].rearrange("(b n) f -> n b f", n=128),
                )

                for bk in pst["buckets"]:
                    q = bk["q"]
                    nt = bk["ntiles"]
                    if nt == 0:
                        continue
                    t0 = bk["tile0"]
                    gats = []
                    pos = 0
                    while pos < nt:
                        m = min(NIDX_TILES, nt - pos)
                        gat = pgp.tile([128, NIDX_TILES * 128], bf16, tag="pgat")
                        nc.gpsimd.dma_gather(
                            gat[:, : m * 128].rearrange("p (t d) -> p t d", d=128),
                            tab2_q[q][:, :],
                            pidx[:, (t0 + pos) * 8 : (t0 + pos + m) * 8],
                            m * 128,
                            m * 128,
                            D,
                            single_packet=False,
                            queue_num=next_queue(),
                        )
                        gats.append((pos, m, gat))
                        pos += m
                    ztiles = {}
                    ARB = 16
                    abf = None
                    for li, mm in enumerate(bk["mms"]):
                        t = mm["tile"]
                        if t not in ztiles:
                            ztiles[t] = zap.tile([128, 128], f32, tag="pza", name=f"pza_{t}")
                        if li % ARB == 0:
                            n = min(ARB, bk["nmm"] - li)
                            abf = parp.tile([1, ARB * 128], bf16, tag="par",
                                            name="parbuf")
                            mi0 = bk["mm0"] + li
                            nc.sync.dma_start(
                                abf[:, : n * 128],
                                parow_t[:, mi0 * 128 : (mi0 + n) * 128],
                            )
                        lo = (li % ARB) * 128
                        ar_ps = arbp.tile([128, 128], f32, tag="parp")
                        nc.tensor.matmul(
                            ar_ps[:, :], lhsT=ones1[:, :],
                            rhs=abf[:, lo : lo + 128],
                            start=True, stop=True,
                        )
                        ex = pxp.tile([128, 128], bf16, tag="pex")
                        nc.vector.tensor_tensor(
                            out=ex[:, :],
                            in0=iotat[:, :1].to_broadcast([128, 128]),
                            in1=ar_ps[:, :],
                            op=eq,
                        )
                        nc.tensor.matmul(
                            ztiles[t][:, :],
                            lhsT=ex[:, :],
                            rhs=h2loc[:, mm["blk"], :],
                            start=mm["start"],
                            stop=mm["stop"],
                        )
                    for base in range(0, nt, 4):
                        nb_ = min(4, nt - base)
                        zt_ps = ztp.tile([128, 512], bf16, tag="pzt")
                        zsb = pzp.tile([128, 512], bf16, tag="pz")
                        for i in range(nb_):
                            t = base + i
                            gat = None
                            for (p0, m, gg) in gats:
                                if p0 <= t < p0 + m:
                                    gat = gg[:, (t - p0) * 128 : (t - p0 + 1) * 128]
                                    break
                            nc.vector.tensor_mul(
                                zsb[:, i * 128 : (i + 1) * 128],
                                ztiles[t][:, :],
                                gat,
                            )
                        for i in range(nb_):
                            nc.tensor.matmul(
                                zt_ps[:, i * 128 : (i + 1) * 128],
                                lhsT=zsb[:, i * 128 : (i + 1) * 128],
                                rhs=identb[:, :],
                                is_transpose=True,
                                start=(i == 0),
                                stop=(i == nb_ - 1),
                            )
                        zt = pep.tile([128, 512], bf16, tag="pzt_s")
                        nc.scalar.copy(zt[:, : nb_ * 128], zt_ps[:, : nb_ * 128])
                        za_ps = mmp.tile([128, 512], f32, tag="pmm")
                        for i in range(nb_):
                            nc.tensor.matmul(
                                za_ps[:, i * 128 : (i + 1) * 128],
                                lhsT=wab[:, :],
                                rhs=zt[:, i * 128 : (i + 1) * 128],
                                start=(i == 0),
                                stop=(i == nb_ - 1),
                            )
                        za = pep.tile([128, 512], bf16, tag="pza_s")
                        nc.scalar.activation(
                            za[:, : nb_ * 128], za_ps[:, : nb_ * 128],
                            mybir.ActivationFunctionType.Relu, bias=bat[:, :],
                        )
                        sc_ps = scp.tile([1, 512], f32, tag="psc")
                        for i in range(nb_):
                            nc.tensor.matmul(
                                sc_ps[:, i * 128 : (i + 1) * 128],
                                lhsT=wbb[:, :],
                                rhs=za[:, i * 128 : (i + 1) * 128],
                                start=(i == 0),
                                stop=(i == nb_ - 1),
                            )
                        sc = pep.tile([1, 512], f32, tag="psc_s")
                        nc.scalar.activation(
                            sc[:, : nb_ * 128], sc_ps[:, : nb_ * 128],
                            mybir.ActivationFunctionType.Identity, bias=float(bb_val),
                        )
                        o0 = (t0 + base) * 128
                        nc.sync.dma_start(
                            out_t[o0 : o0 + nb_ * 128].rearrange("(x n) -> x n", x=1),
                            sc[:, : nb_ * 128],
                        )
    nc.compile()
    return nc


# ------------------------------------------------------------------ entry
def kernel(node_ids, senders, receivers, pairs, emb, W1, b1, W2, b2, Wa, ba,
           Wb, bb):
    global _LAST_EXEC_NS, _LAST_RESULTS
    import ml_dtypes
    from concourse import bass_utils

    bf = ml_dtypes.bfloat16
    node_ids = np.asarray(node_ids).astype(np.int64)
    senders = np.asarray(senders).astype(np.int64)
    receivers = np.asarray(receivers).astype(np.int64)
    pairs_np = np.asarray(pairs).astype(np.int64)
    emb = np.asarray(emb, dtype=np.float32)
    W1 = np.asarray(W1, dtype=np.float32)
    b1 = np.asarray(b1, dtype=np.float32)
    W2 = np.asarray(W2, dtype=np.float32)
    b2 = np.asarray(b2, dtype=np.float32)
    Wa = np.asarray(Wa, dtype=np.float32)
    ba = np.asarray(ba, dtype=np.float32)
    Wb = np.asarray(Wb, dtype=np.float32)
    bb = np.asarray(bb, dtype=np.float32)

    N = emb.shape[0]
    cfg = Cfg(N)
    x0 = emb[node_ids]

    est, edata = _plan_edges(cfg, senders, receivers)
    pst, pdata = _plan_pairs(cfg, pairs_np)
    ssend, srecv = _norms(cfg, senders, receivers)

    xn0 = (x0 * ssend[:, None]).astype(bf)
    tab0 = _chunkify(cfg, _shard_pad(cfg, xn0))
    xn0l = _shard_pad(cfg, xn0)
    x0l = _shard_pad(cfg, x0.astype(bf))
    ssend_sh = _shard_pad(cfg, ssend)
    srecv_sh = _shard_pad(cfg, srecv)

    nc = _build(cfg, est, pst, float(bb.reshape(-1)[0]))

    iota = np.tile(np.arange(128, dtype=np.float32), (128, 1))
    iotat = np.ascontiguousarray(iota.T)
    in_maps = []
    for k in range(R):
        in_maps.append(
            dict(
                **{f"tab0_{q}": tab0[q] for q in range(cfg.NQ)},
                xn0l=xn0l[k],
                x0l=x0l[k],
                eidx=edata["eidx"][k],
                eseg=edata["eseg"][k],
                pidx=pdata["pidx"][k],
                parow=pdata["parow"][k],
                ssend=ssend_sh[k],
                srecv=srecv_sh[k],
                w1t=np.ascontiguousarray(W1[:D]),
                w1b=np.ascontiguousarray(W1[D:]),
                w2t=np.ascontiguousarray(W2[:D]),
                w2b=np.ascontiguousarray(W2[D:]),
                wa=Wa,
                wb=Wb,
                b1=b1.reshape(1, D),
                b2=b2.reshape(1, D),
                ba=ba.reshape(D, 1),
                iota=iota,
                iotat=iotat,
            )
        )

    res = bass_utils.run_bass_kernel_spmd(
        nc, in_maps, core_ids=list(range(R)), trace=_TRACE
    )
    _LAST_EXEC_NS = res.exec_time_ns
    _LAST_RESULTS = res

    P = pairs_np.shape[0]
    scores = np.zeros(P, np.float32)
    for k in range(R):
        v = np.asarray(res.results[k]["scores"])
        pm = pdata["posmap"][k]
        m = pm >= 0
        scores[pm[m]] = v[m]
    return scores
